# revision 3
# baseline (speedup 1.0000x reference)
"""Trainium2 Bass kernel for cross "efficient attention".

Reference computation (per batch b, head h, with C=128, HEADS=8, hc=16, n=16384):
    k = x2[b].reshape(HEADS, hc, n); v = x1[b].reshape(HEADS, hc, n)
    key_sm   = softmax(k, axis=-1)          # over n
    query_sm = softmax(k, axis=1)           # over hc (head channels)
    context  = key_sm @ v^T                 # (hc, hc)
    out[b,h] = context^T @ query_sm         # (hc, n)

Sharding: data-parallel over batch B=8 across the 8 NeuronCores (no
collectives).  Inputs are cast to bf16 on the host (tolerance is 2e-2;
bf16 end-to-end measures ~5e-3), halving HBM traffic, and x1 is laid
out host-side as [128, N/128, C] blocks so every DMA descriptor is a
contiguous 4 KiB run.

Per-core pipeline (N = 16384 in 8 slabs of 2048 = 16 chunks of 128):
  pass 1 (per chunk j):
    MM_t: matmul(lhsT=exp_chunk, rhs=[I|ind8]) -> PSUM f32 [128,136]
          cols 0:128 = exp^T chunk, cols 128:136 = per-head colsums
    scalar/vector copy PSUM -> eT slab buffer (bf16 cast)
    MM_ctx (one slab lag): ctx += eT_chunk^T @ vT_chunk (PSUM f32)
  per slab: vector strided-copies colsums -> f32, reciprocal_approx_fast,
    gpsimd broadcasts recip [128,16,8] -> rbc [128,2048] bf16 (head-major)
  bd = (ctx / rowsum) * blockdiag_mask  (bf16 [C,C])
  pass 2 (per slab, chunk j): MM_att: matmul(lhsT=exp_chunk, rhs=bd) ->
    attT PSUM f32 [128, 2048]; normalize att*rbc alternating between
    vector (direct tensor_mul) and scalar-copy + gpsimd-multiply,
    -> bf16 out tile -> DMA out.
Output leaves the device transposed ([128, N/128, C] blocks); the host
reassembles [C, H, W].
"""

import numpy as np
from contextlib import ExitStack

B, C, H, W = 8, 128, 128, 128
N = H * W                 # 16384
HEADS, HC = 8, 16
NCORES = 8
SLAB = 2048               # slab width (16 chunks of 128)
NCH = SLAB // C           # chunks per slab = 16
NSLAB = N // SLAB         # 8
NB = N // C               # 128 chunk-blocks total
MW = C + HEADS            # 136: movmat cols = [I | ind8]
SCAL_COPIES = 5           # chunks per slab whose PSUM->SBUF copy runs on scalar

_cache: dict = {}


def _build():
    import concourse.bass as bass
    import concourse.tile as tile
    from concourse import bacc, mybir

    FP32 = mybir.dt.float32
    BF16 = mybir.dt.bfloat16
    AF = mybir.ActivationFunctionType

    nc = bacc.Bacc("TRN2", target_bir_lowering=False, debug=False)

    x1t_d = nc.dram_tensor("x1t", [C, NB, C], BF16, kind="ExternalInput")
    x2_d = nc.dram_tensor("x2", [C, N], BF16, kind="ExternalInput")
    mov_d = nc.dram_tensor("movmat", [C, MW], BF16, kind="ExternalInput")
    bd8_d = nc.dram_tensor("bd8", [C, C], BF16, kind="ExternalInput")
    out_d = nc.dram_tensor("out", [C, NB, C], BF16, kind="ExternalOutput")

    with tile.TileContext(nc) as tc:
        with ExitStack() as ctx:
            persist = ctx.enter_context(tc.tile_pool(name="persist", bufs=1))
            x2ld = ctx.enter_context(tc.tile_pool(name="x2ld", bufs=3))
            vTp = ctx.enter_context(tc.tile_pool(name="vTp", bufs=3))
            eTp = ctx.enter_context(tc.tile_pool(name="eTp", bufs=2))
            rcpp = ctx.enter_context(tc.tile_pool(name="rcpp", bufs=2))
            outp = ctx.enter_context(tc.tile_pool(name="outp", bufs=2))
            qtmp = ctx.enter_context(tc.tile_pool(name="qtmp", bufs=2))
            smalls = ctx.enter_context(tc.tile_pool(name="smalls", bufs=1))

            exp_nat = persist.tile([C, N], BF16, tag="exp_nat")
            rbc = persist.tile([C, N], BF16, tag="rbc")      # bcast recip
            rs_acc = smalls.tile([C, NSLAB], FP32, tag="rs_acc")
            movmat = smalls.tile([C, MW], BF16, tag="movmat")
            bd8 = smalls.tile([C, C], BF16, tag="bd8")
            bd = smalls.tile([C, C], BF16, tag="bd")

            with tc.tile_pool(name="psctx", bufs=1, space="PSUM") as ps_ctx, \
                 tc.tile_pool(name="pstre", bufs=4, space="PSUM") as ps_te:
                ctx_ps = ps_ctx.tile([C, C], FP32, tag="ctx")

                n_mm = NB
                mm_idx = 0
                pending = None   # (eT, vT) of the previous slab

                def emit_ctx(eT, vT):
                    nonlocal mm_idx
                    eTv = eT[:].rearrange("p (j w) -> p j w", w=MW)
                    vTv = vT[:].rearrange("p (j c) -> p j c", c=C)
                    for j in range(NCH):
                        nc.tensor.matmul(
                            ctx_ps[:],
                            eTv[:, j, 0:C],     # lhsT: (n0=128, c_k=128)
                            vTv[:, j, :],       # rhs : (n0=128, c_v=128)
                            start=(mm_idx == 0),
                            stop=(mm_idx == n_mm - 1),
                        )
                        mm_idx += 1

                for i in range(NSLAB):
                    sl = bass.ds(i * SLAB, SLAB)
                    x2t = x2ld.tile([C, SLAB], BF16, tag="x2t")
                    nc.sync.dma_start(out=x2t[:], in_=x2_d[:, sl])
                    vT = vTp.tile([C, SLAB], BF16, tag="vT")
                    nc.sync.dma_start(
                        out=vT[:].rearrange("p (j c) -> p j c", c=C),
                        in_=x1t_d[:, bass.ds(i * NCH, NCH), :],
                    )
                    if i == 0:
                        nc.sync.dma_start(out=movmat[:], in_=mov_d[:])
                        nc.sync.dma_start(out=bd8[:], in_=bd8_d[:])

                    nc.scalar.activation(
                        exp_nat[:, sl], x2t[:], AF.Exp,
                        accum_out=rs_acc[:, i:i + 1],
                    )

                    # transpose + colsum: one matmul per chunk, copy out
                    # (bf16 cast) split between scalar and vector
                    eT = eTp.tile([C, NCH * MW], BF16, tag="eT")
                    for j in range(NCH):
                        te = ps_te.tile([C, MW], FP32, tag="te")
                        nc.tensor.matmul(
                            te[:],
                            exp_nat[:, bass.ds(i * SLAB + j * C, C)],
                            movmat[:],
                        )
                        eng = nc.scalar.copy if j < SCAL_COPIES \
                            else nc.vector.tensor_copy
                        eng(eT[:, bass.ds(j * MW, MW)], te[:])

                    # per-slab: colsums -> f32 -> recip -> bcast bf16
                    csf = rcpp.tile([C, NCH * HEADS], FP32, tag="csf")
                    eTv = eT[:].rearrange("p (j w) -> p j w", w=MW)
                    nc.vector.tensor_copy(
                        csf[:].rearrange("p (j h) -> p j h", h=HEADS),
                        eTv[:, :, C:MW],
                    )
                    rcp = rcpp.tile([C, NCH * HEADS], FP32, tag="rcp")
                    nc.vector.reciprocal_approx_fast(out=rcp[:], in_=csf[:])
                    nc.gpsimd.tensor_copy(
                        rbc[:, sl].rearrange("p (j h c) -> p j h c", h=HEADS, c=HC),
                        rcp[:].rearrange("p (j h) -> p j h", h=HEADS)
                        .broadcast_to([C, NCH, HEADS, HC]),
                    )

                    if pending is not None:
                        emit_ctx(*pending)
                    pending = (eT, vT)
                emit_ctx(*pending)

                # ---- block-diagonal context weights ----
                rowsum = smalls.tile([C, 1], FP32, tag="rowsum")
                nc.vector.tensor_reduce(
                    rowsum[:], rs_acc[:], mybir.AxisListType.X, mybir.AluOpType.add
                )
                rs_rcp = smalls.tile([C, 1], FP32, tag="rs_rcp")
                nc.vector.reciprocal(rs_rcp[:], rowsum[:])
                scaled = smalls.tile([C, C], BF16, tag="scaled")
                nc.vector.tensor_scalar(
                    scaled[:], ctx_ps[:], rs_rcp[:, 0:1], None, mybir.AluOpType.mult
                )
                nc.vector.tensor_mul(bd[:], scaled[:], bd8[:])

            # ---- pass 2: attended (transposed), normalize, store ----
            with tc.tile_pool(name="psatt", bufs=2, space="PSUM") as ps_att:
                for i in range(NSLAB):
                    sl = bass.ds(i * SLAB, SLAB)
                    att = ps_att.tile([C, SLAB], FP32, tag="att")
                    for j in range(NCH):
                        nc.tensor.matmul(
                            att[:, bass.ds(j * C, C)],
                            exp_nat[:, bass.ds(i * SLAB + j * C, C)],
                            bd[:],
                        )
                    ot = outp.tile([C, SLAB], BF16, tag="ot")
                    if i % 2 == 0:
                        nc.vector.tensor_mul(ot[:], att[:], rbc[:, sl])
                    else:
                        qt = qtmp.tile([C, SLAB], BF16, tag="qt")
                        nc.scalar.copy(qt[:], att[:])
                        nc.gpsimd.tensor_mul(ot[:], qt[:], rbc[:, sl])
                    nc.sync.dma_start(
                        out=out_d[:, bass.ds(i * NCH, NCH), :],
                        in_=ot[:].rearrange("p (j c) -> p j c", c=C),
                    )

    nc.compile()
    return nc


def _get_nc():
    if "nc" not in _cache:
        _cache["nc"] = _build()
    return _cache["nc"]


def _consts_np():
    import ml_dtypes

    mov = np.zeros((C, MW), dtype=np.float32)
    mov[:C, :C] = np.eye(C, dtype=np.float32)
    for h in range(HEADS):
        mov[h * HC:(h + 1) * HC, C + h] = 1.0
    bd8 = np.zeros((C, C), dtype=np.float32)
    for h in range(HEADS):
        bd8[h * HC:(h + 1) * HC, h * HC:(h + 1) * HC] = 1.0
    return mov.astype(ml_dtypes.bfloat16), bd8.astype(ml_dtypes.bfloat16)


def _to_np(a) -> np.ndarray:
    """Materialize to float32 numpy; retry once on a transient bad fetch
    (device-backed arrays have been observed to materialize NaNs once)."""
    out = np.asarray(a, dtype=np.float32)
    if np.isnan(out).any():
        out = np.asarray(a, dtype=np.float32)
    return out


def make_in_maps(x1: np.ndarray, x2: np.ndarray):
    import ml_dtypes

    bf16 = ml_dtypes.bfloat16
    x1 = _to_np(x1).reshape(B, C, N)
    x2 = _to_np(x2).reshape(B, C, N)
    # x1 blocked-transposed: x1t[b, p, j, c] = x1[b, c, j*128 + p]
    x1t = np.ascontiguousarray(
        x1.reshape(B, C, NB, C).transpose(0, 3, 2, 1)
    ).astype(bf16)
    x2b = x2.astype(bf16)
    mov, bd8 = _consts_np()
    return [
        {"x1t": x1t[i], "x2": x2b[i], "movmat": mov, "bd8": bd8}
        for i in range(NCORES)
    ]


def kernel(x1: np.ndarray, x2: np.ndarray) -> np.ndarray:
    from concourse.bass_utils import run_bass_kernel_spmd

    nc = _get_nc()
    in_maps = make_in_maps(x1, x2)
    res = run_bass_kernel_spmd(nc, in_maps, core_ids=list(range(NCORES)))
    outs = []
    for i in range(NCORES):
        o = np.asarray(res.results[i]["out"], dtype=np.float32)  # [128, NB, C]
        outs.append(o.transpose(2, 1, 0).reshape(C, N))          # [C, N]
    return np.stack(outs, axis=0).reshape(B, C, H, W)


# revision 4
# speedup vs baseline: 1.3549x; 1.3549x over previous
"""Trainium2 Bass kernel for cross "efficient attention".

Reference computation (per batch b, head h, with C=128, HEADS=8, hc=16, n=16384):
    k = x2[b].reshape(HEADS, hc, n); v = x1[b].reshape(HEADS, hc, n)
    key_sm   = softmax(k, axis=-1)          # over n
    query_sm = softmax(k, axis=1)           # over hc (head channels)
    context  = key_sm @ v^T                 # (hc, hc)
    out[b,h] = context^T @ query_sm         # (hc, n)

Sharding: data-parallel over batch B=8 across the 8 NeuronCores (no
collectives).  Inputs are cast to bf16 on the host (tolerance is 2e-2;
bf16 end-to-end measures ~5e-3), halving HBM traffic, and x1 is laid
out host-side as [128, N/128, C] blocks so every DMA descriptor is a
contiguous 4 KiB run.

Per-core pipeline (N = 16384 in 8 slabs of 2048 = 16 chunks of 128):
  pass 1 (per chunk j):
    MM_t : transpose-mode matmul -> te PSUM bf16 [128,128] (exp^T chunk)
    MM_cs: matmul(lhsT=exp_chunk, rhs=ind8) -> per-slab PSUM f32
           accumulator [128, 16*8] (per-head colsums, transposed layout)
    scalar/vector copy te -> eT slab buffer (bf16->bf16, fast path)
    MM_ctx (one slab lag): ctx += eT_chunk^T @ vT_chunk (PSUM f32)
  per slab: one reciprocal_approx_fast [128,128] PSUM->SBUF (rcp_all)
  bd = (ctx / rowsum) * blockdiag_mask  (bf16 [C,C])
  pass 2 (per slab, chunk j): MM_att: matmul(lhsT=exp_chunk, rhs=bd) ->
    attT PSUM f32 [128, 2048]; one vector tensor_mul per slab with the
    per-head reciprocals broadcast via a stride-0 AP -> bf16 out tile
    -> DMA out.
Output leaves the device transposed ([128, N/128, C] blocks); the host
reassembles [C, H, W].
"""

import numpy as np
from contextlib import ExitStack

B, C, H, W = 8, 128, 128, 128
N = H * W                 # 16384
HEADS, HC = 8, 16
NCORES = 8
SLAB = 2048               # slab width (16 chunks of 128)
NCH = SLAB // C           # chunks per slab = 16
NSLAB = N // SLAB         # 8
NB = N // C               # 128 chunk-blocks total
SCAL_COPIES = 10          # chunks per slab whose PSUM->SBUF copy runs on scalar

_cache: dict = {}


def _build():
    import concourse.bass as bass
    import concourse.tile as tile
    from concourse import bacc, mybir

    FP32 = mybir.dt.float32
    BF16 = mybir.dt.bfloat16
    AF = mybir.ActivationFunctionType

    nc = bacc.Bacc("TRN2", target_bir_lowering=False, debug=False)

    x1t_d = nc.dram_tensor("x1t", [C, NB, C], BF16, kind="ExternalInput")
    x2_d = nc.dram_tensor("x2", [C, N], BF16, kind="ExternalInput")
    id_d = nc.dram_tensor("ident", [C, C], BF16, kind="ExternalInput")
    ind8_d = nc.dram_tensor("ind8", [C, HEADS], BF16, kind="ExternalInput")
    bd8_d = nc.dram_tensor("bd8", [C, C], BF16, kind="ExternalInput")
    out_d = nc.dram_tensor("out", [C, NB, C], BF16, kind="ExternalOutput")

    with tile.TileContext(nc) as tc:
        with ExitStack() as ctx:
            persist = ctx.enter_context(tc.tile_pool(name="persist", bufs=1))
            x2ld = ctx.enter_context(tc.tile_pool(name="x2ld", bufs=3))
            vTp = ctx.enter_context(tc.tile_pool(name="vTp", bufs=3))
            eTp = ctx.enter_context(tc.tile_pool(name="eTp", bufs=2))
            outp = ctx.enter_context(tc.tile_pool(name="outp", bufs=2))
            smalls = ctx.enter_context(tc.tile_pool(name="smalls", bufs=1))

            exp_nat = persist.tile([C, N], BF16, tag="exp_nat")
            rcp_all = persist.tile([C, NB * HEADS], FP32, tag="rcp_all")
            rs_acc = smalls.tile([C, NSLAB], FP32, tag="rs_acc")
            ident = smalls.tile([C, C], BF16, tag="ident")
            ind8 = smalls.tile([C, HEADS], BF16, tag="ind8")
            bd8 = smalls.tile([C, C], BF16, tag="bd8")
            bd = smalls.tile([C, C], BF16, tag="bd")

            with tc.tile_pool(name="psctx", bufs=1, space="PSUM") as ps_ctx, \
                 tc.tile_pool(name="pstre", bufs=4, space="PSUM") as ps_te, \
                 tc.tile_pool(name="pscs", bufs=2, space="PSUM") as ps_cs:
                ctx_ps = ps_ctx.tile([C, C], FP32, tag="ctx")

                n_mm = NB
                mm_idx = 0
                pending = None   # (eT, vT) of the previous slab

                def emit_ctx(eT, vT):
                    nonlocal mm_idx
                    eTv = eT[:].rearrange("p (j c) -> p j c", c=C)
                    vTv = vT[:].rearrange("p (j c) -> p j c", c=C)
                    for j in range(NCH):
                        nc.tensor.matmul(
                            ctx_ps[:],
                            eTv[:, j, :],       # lhsT: (n0=128, c_k=128)
                            vTv[:, j, :],       # rhs : (n0=128, c_v=128)
                            start=(mm_idx == 0),
                            stop=(mm_idx == n_mm - 1),
                        )
                        mm_idx += 1

                for i in range(NSLAB):
                    sl = bass.ds(i * SLAB, SLAB)
                    x2t = x2ld.tile([C, SLAB], BF16, tag="x2t")
                    nc.sync.dma_start(out=x2t[:], in_=x2_d[:, sl])
                    vT = vTp.tile([C, SLAB], BF16, tag="vT")
                    nc.sync.dma_start(
                        out=vT[:].rearrange("p (j c) -> p j c", c=C),
                        in_=x1t_d[:, bass.ds(i * NCH, NCH), :],
                    )
                    if i == 0:
                        nc.sync.dma_start(out=ident[:], in_=id_d[:])
                        nc.sync.dma_start(out=ind8[:], in_=ind8_d[:])
                        nc.sync.dma_start(out=bd8[:], in_=bd8_d[:])

                    nc.scalar.activation(
                        exp_nat[:, sl], x2t[:], AF.Exp,
                        accum_out=rs_acc[:, i:i + 1],
                    )

                    # per-chunk: transpose (bf16) + colsum (f32 accum tile)
                    eT = eTp.tile([C, SLAB], BF16, tag="eT")
                    cs_ps = ps_cs.tile([C, NCH * HEADS], FP32, tag="cs")
                    for j in range(NCH):
                        e_chunk = exp_nat[:, bass.ds(i * SLAB + j * C, C)]
                        te = ps_te.tile([C, C], BF16, tag="te")
                        nc.tensor.transpose(te[:], e_chunk, ident[:])
                        nc.tensor.matmul(
                            cs_ps[:, bass.ds(j * HEADS, HEADS)],
                            e_chunk, ind8[:],
                        )
                        eng = nc.scalar.copy if j < SCAL_COPIES \
                            else nc.vector.tensor_copy
                        eng(eT[:, bass.ds(j * C, C)], te[:])

                    # per-slab: one reciprocal over the colsum accumulator
                    nc.vector.reciprocal_approx_fast(
                        out=rcp_all[:, bass.ds(i * NCH * HEADS, NCH * HEADS)],
                        in_=cs_ps[:],
                    )

                    if pending is not None:
                        emit_ctx(*pending)
                    pending = (eT, vT)
                emit_ctx(*pending)

                # ---- block-diagonal context weights ----
                rowsum = smalls.tile([C, 1], FP32, tag="rowsum")
                nc.vector.tensor_reduce(
                    rowsum[:], rs_acc[:], mybir.AxisListType.X, mybir.AluOpType.add
                )
                rs_rcp = smalls.tile([C, 1], FP32, tag="rs_rcp")
                nc.vector.reciprocal(rs_rcp[:], rowsum[:])
                scaled = smalls.tile([C, C], BF16, tag="scaled")
                nc.vector.tensor_scalar(
                    scaled[:], ctx_ps[:], rs_rcp[:, 0:1], None, mybir.AluOpType.mult
                )
                nc.vector.tensor_mul(bd[:], scaled[:], bd8[:])

            # ---- pass 2: attended (transposed), normalize, store ----
            with tc.tile_pool(name="psatt", bufs=2, space="PSUM") as ps_att:
                for i in range(NSLAB):
                    att = ps_att.tile([C, SLAB], FP32, tag="att")
                    for j in range(NCH):
                        nc.tensor.matmul(
                            att[:, bass.ds(j * C, C)],
                            exp_nat[:, bass.ds(i * SLAB + j * C, C)],
                            bd[:],
                        )
                    ot = outp.tile([C, SLAB], BF16, tag="ot")
                    nc.vector.tensor_mul(
                        ot[:].rearrange("p (j h c) -> p j h c", h=HEADS, c=HC),
                        att[:].rearrange("p (j h c) -> p j h c", h=HEADS, c=HC),
                        rcp_all[:, bass.ds(i * NCH * HEADS, NCH * HEADS)]
                        .rearrange("p (j h) -> p j h", h=HEADS)
                        .broadcast_to([C, NCH, HEADS, HC]),
                    )
                    nc.sync.dma_start(
                        out=out_d[:, bass.ds(i * NCH, NCH), :],
                        in_=ot[:].rearrange("p (j c) -> p j c", c=C),
                    )

    nc.compile()
    return nc


def _get_nc():
    if "nc" not in _cache:
        _cache["nc"] = _build()
    return _cache["nc"]


def _consts_np():
    import ml_dtypes

    bf16 = ml_dtypes.bfloat16
    ident = np.eye(C, dtype=np.float32).astype(bf16)
    ind8 = np.zeros((C, HEADS), dtype=np.float32)
    for h in range(HEADS):
        ind8[h * HC:(h + 1) * HC, h] = 1.0
    bd8 = np.zeros((C, C), dtype=np.float32)
    for h in range(HEADS):
        bd8[h * HC:(h + 1) * HC, h * HC:(h + 1) * HC] = 1.0
    return ident, ind8.astype(bf16), bd8.astype(bf16)


def _to_np(a) -> np.ndarray:
    """Materialize to float32 numpy; retry once on a transient bad fetch
    (device-backed arrays have been observed to materialize NaNs once)."""
    out = np.asarray(a, dtype=np.float32)
    if np.isnan(out).any():
        out = np.asarray(a, dtype=np.float32)
    return out


def make_in_maps(x1: np.ndarray, x2: np.ndarray):
    import ml_dtypes

    bf16 = ml_dtypes.bfloat16
    x1 = _to_np(x1).reshape(B, C, N)
    x2 = _to_np(x2).reshape(B, C, N)
    # x1 blocked-transposed: x1t[b, p, j, c] = x1[b, c, j*128 + p]
    x1t = np.ascontiguousarray(
        x1.reshape(B, C, NB, C).transpose(0, 3, 2, 1)
    ).astype(bf16)
    x2b = x2.astype(bf16)
    ident, ind8, bd8 = _consts_np()
    return [
        {"x1t": x1t[i], "x2": x2b[i], "ident": ident, "ind8": ind8, "bd8": bd8}
        for i in range(NCORES)
    ]


def kernel(x1: np.ndarray, x2: np.ndarray) -> np.ndarray:
    from concourse.bass_utils import run_bass_kernel_spmd

    nc = _get_nc()
    in_maps = make_in_maps(x1, x2)
    res = run_bass_kernel_spmd(nc, in_maps, core_ids=list(range(NCORES)))
    outs = []
    for i in range(NCORES):
        o = np.asarray(res.results[i]["out"], dtype=np.float32)  # [128, NB, C]
        outs.append(o.transpose(2, 1, 0).reshape(C, N))          # [C, N]
    return np.stack(outs, axis=0).reshape(B, C, H, W)


# revision 6
# speedup vs baseline: 1.5308x; 1.1298x over previous
"""Trainium2 Bass kernel for cross "efficient attention".

Reference computation (per batch b, head h, with C=128, HEADS=8, hc=16, n=16384):
    k = x2[b].reshape(HEADS, hc, n); v = x1[b].reshape(HEADS, hc, n)
    key_sm   = softmax(k, axis=-1)          # over n
    query_sm = softmax(k, axis=1)           # over hc (head channels)
    context  = key_sm @ v^T                 # (hc, hc)
    out[b,h] = context^T @ query_sm         # (hc, n)

Sharding: data-parallel over batch B=8 across the 8 NeuronCores (no
collectives).  Inputs are cast to bf16 on the host (tolerance is 2e-2;
bf16 end-to-end measures ~5e-3), halving HBM traffic, and x1 is laid
out host-side as [128, N/128, C] blocks so every DMA descriptor is a
contiguous 4 KiB run.

Per-core pipeline (N = 16384 in 8 slabs of 2048 = 16 chunks of 128):
  pass 1 (per chunk j):
    MM_t : transpose-mode matmul -> te PSUM bf16 [128,128] (exp^T chunk)
    MM_cs: matmul(lhsT=exp_chunk, rhs=ind8) -> per-slab PSUM f32
           accumulator [128, 16*8] (per-head colsums, transposed layout)
    scalar/vector copy te -> eT slab buffer (bf16->bf16, fast path)
    MM_ctx (one slab lag): ctx += eT_chunk^T @ vT_chunk (PSUM f32)
  per slab: one reciprocal_approx_fast [128,128] PSUM->SBUF (rcp_all)
  bd = (ctx / rowsum) * blockdiag_mask  (bf16 [C,C])
  pass 2 (per slab, chunk j): MM_att: matmul(lhsT=exp_chunk, rhs=bd) ->
    attT PSUM f32 [128, 2048]; one vector tensor_mul per slab with the
    per-head reciprocals broadcast via a stride-0 AP -> bf16 out tile
    -> DMA out.
Output leaves the device transposed ([128, N/128, C] blocks); the host
reassembles [C, H, W].
"""

import numpy as np
from contextlib import ExitStack

B, C, H, W = 8, 128, 128, 128
N = H * W                 # 16384
HEADS, HC = 8, 16
NCORES = 8
SLAB = 2048               # slab width (16 chunks of 128)
NCH = SLAB // C           # chunks per slab = 16
NSLAB = N // SLAB         # 8
NB = N // C               # 128 chunk-blocks total
GRP = 8                   # transpose chunks batched per PSUM group tile
NGRP = NCH // GRP         # copy groups per slab = 2

_cache: dict = {}


def _build():
    import concourse.bass as bass
    import concourse.tile as tile
    from concourse import bacc, mybir

    FP32 = mybir.dt.float32
    BF16 = mybir.dt.bfloat16
    AF = mybir.ActivationFunctionType

    nc = bacc.Bacc("TRN2", target_bir_lowering=False, debug=False)

    x1t_d = nc.dram_tensor("x1t", [C, NB, C], BF16, kind="ExternalInput")
    x2_d = nc.dram_tensor("x2", [C, N], BF16, kind="ExternalInput")
    id_d = nc.dram_tensor("ident", [C, C], BF16, kind="ExternalInput")
    ind8_d = nc.dram_tensor("ind8", [C, HEADS], BF16, kind="ExternalInput")
    bd8_d = nc.dram_tensor("bd8", [C, C], BF16, kind="ExternalInput")
    out_d = nc.dram_tensor("out", [C, NB, C], BF16, kind="ExternalOutput")

    with tile.TileContext(nc) as tc:
        with ExitStack() as ctx:
            persist = ctx.enter_context(tc.tile_pool(name="persist", bufs=1))
            x2ld = ctx.enter_context(tc.tile_pool(name="x2ld", bufs=3))
            vTp = ctx.enter_context(tc.tile_pool(name="vTp", bufs=3))
            eTp = ctx.enter_context(tc.tile_pool(name="eTp", bufs=2))
            outp = ctx.enter_context(tc.tile_pool(name="outp", bufs=2))
            smalls = ctx.enter_context(tc.tile_pool(name="smalls", bufs=1))

            exp_nat = persist.tile([C, N], BF16, tag="exp_nat")
            rcp_all = persist.tile([C, NB * HEADS], FP32, tag="rcp_all")
            rs_acc = smalls.tile([C, NSLAB], FP32, tag="rs_acc")
            ident = smalls.tile([C, C], BF16, tag="ident")
            ind8 = smalls.tile([C, HEADS], BF16, tag="ind8")
            bd8 = smalls.tile([C, C], BF16, tag="bd8")
            bd = smalls.tile([C, C], BF16, tag="bd")

            with tc.tile_pool(name="psctx", bufs=1, space="PSUM") as ps_ctx, \
                 tc.tile_pool(name="pstre", bufs=4, space="PSUM") as ps_te, \
                 tc.tile_pool(name="pscs", bufs=2, space="PSUM") as ps_cs:
                ctx_ps = ps_ctx.tile([C, C], FP32, tag="ctx")

                n_mm = NB
                mm_idx = 0
                pending = None   # (eT, vT) of the previous slab

                def emit_ctx(eT, vT):
                    nonlocal mm_idx
                    eTv = eT[:].rearrange("p (j c) -> p j c", c=C)
                    vTv = vT[:].rearrange("p (j c) -> p j c", c=C)
                    for j in range(NCH):
                        nc.tensor.matmul(
                            ctx_ps[:],
                            eTv[:, j, :],       # lhsT: (n0=128, c_k=128)
                            vTv[:, j, :],       # rhs : (n0=128, c_v=128)
                            start=(mm_idx == 0),
                            stop=(mm_idx == n_mm - 1),
                        )
                        mm_idx += 1

                nc.sync.dma_start(out=ident[:], in_=id_d[:])
                nc.sync.dma_start(out=ind8[:], in_=ind8_d[:])
                nc.sync.dma_start(out=bd8[:], in_=bd8_d[:])
                for i in range(NSLAB):
                    sl = bass.ds(i * SLAB, SLAB)
                    x2t = x2ld.tile([C, SLAB], BF16, tag="x2t")
                    nc.sync.dma_start(out=x2t[:], in_=x2_d[:, sl])
                    vT = vTp.tile([C, SLAB], BF16, tag="vT")
                    nc.sync.dma_start(
                        out=vT[:].rearrange("p (j c) -> p j c", c=C),
                        in_=x1t_d[:, bass.ds(i * NCH, NCH), :],
                    )

                    nc.scalar.activation(
                        exp_nat[:, sl], x2t[:], AF.Exp,
                        accum_out=rs_acc[:, i:i + 1],
                    )

                    # per-chunk transpose (bf16, grouped PSUM tiles) +
                    # colsum (f32 accum tile); one wide copy per group
                    eT = eTp.tile([C, SLAB], BF16, tag="eT")
                    cs_ps = ps_cs.tile([C, NCH * HEADS], FP32, tag="cs")
                    for g in range(NGRP):
                        te = ps_te.tile([C, GRP * C], BF16, tag="te")
                        for jj in range(GRP):
                            j = g * GRP + jj
                            e_chunk = exp_nat[:, bass.ds(i * SLAB + j * C, C)]
                            nc.tensor.transpose(
                                te[:, bass.ds(jj * C, C)], e_chunk, ident[:]
                            )
                            nc.tensor.matmul(
                                cs_ps[:, bass.ds(j * HEADS, HEADS)],
                                e_chunk, ind8[:],
                            )
                        nc.vector.tensor_copy(
                            eT[:, bass.ds(g * GRP * C, GRP * C)], te[:]
                        )

                    # per-slab: one reciprocal over the colsum accumulator
                    nc.vector.reciprocal_approx_fast(
                        out=rcp_all[:, bass.ds(i * NCH * HEADS, NCH * HEADS)],
                        in_=cs_ps[:],
                    )

                    if pending is not None:
                        emit_ctx(*pending)
                    pending = (eT, vT)
                emit_ctx(*pending)

                # ---- block-diagonal context weights ----
                rowsum = smalls.tile([C, 1], FP32, tag="rowsum")
                nc.vector.tensor_reduce(
                    rowsum[:], rs_acc[:], mybir.AxisListType.X, mybir.AluOpType.add
                )
                rs_rcp = smalls.tile([C, 1], FP32, tag="rs_rcp")
                nc.vector.reciprocal(rs_rcp[:], rowsum[:])
                scaled = smalls.tile([C, C], BF16, tag="scaled")
                nc.vector.tensor_scalar(
                    scaled[:], ctx_ps[:], rs_rcp[:, 0:1], None, mybir.AluOpType.mult
                )
                nc.vector.tensor_mul(bd[:], scaled[:], bd8[:])

            # ---- pass 2: attended (transposed), normalize, store ----
            with tc.tile_pool(name="psatt", bufs=2, space="PSUM") as ps_att:
                for i in range(NSLAB):
                    att = ps_att.tile([C, SLAB], FP32, tag="att")
                    for j in range(NCH):
                        nc.tensor.matmul(
                            att[:, bass.ds(j * C, C)],
                            exp_nat[:, bass.ds(i * SLAB + j * C, C)],
                            bd[:],
                        )
                    ot = outp.tile([C, SLAB], BF16, tag="ot")
                    nc.vector.tensor_mul(
                        ot[:].rearrange("p (j h c) -> p j h c", h=HEADS, c=HC),
                        att[:].rearrange("p (j h c) -> p j h c", h=HEADS, c=HC),
                        rcp_all[:, bass.ds(i * NCH * HEADS, NCH * HEADS)]
                        .rearrange("p (j h) -> p j h", h=HEADS)
                        .broadcast_to([C, NCH, HEADS, HC]),
                    )
                    nc.sync.dma_start(
                        out=out_d[:, bass.ds(i * NCH, NCH), :],
                        in_=ot[:].rearrange("p (j c) -> p j c", c=C),
                    )

    nc.compile()
    return nc


def _get_nc():
    if "nc" not in _cache:
        _cache["nc"] = _build()
    return _cache["nc"]


def _consts_np():
    import ml_dtypes

    bf16 = ml_dtypes.bfloat16
    ident = np.eye(C, dtype=np.float32).astype(bf16)
    ind8 = np.zeros((C, HEADS), dtype=np.float32)
    for h in range(HEADS):
        ind8[h * HC:(h + 1) * HC, h] = 1.0
    bd8 = np.zeros((C, C), dtype=np.float32)
    for h in range(HEADS):
        bd8[h * HC:(h + 1) * HC, h * HC:(h + 1) * HC] = 1.0
    return ident, ind8.astype(bf16), bd8.astype(bf16)


def _to_np(a) -> np.ndarray:
    """Materialize to float32 numpy; retry once on a transient bad fetch
    (device-backed arrays have been observed to materialize NaNs once)."""
    out = np.asarray(a, dtype=np.float32)
    if np.isnan(out).any():
        out = np.asarray(a, dtype=np.float32)
    return out


def make_in_maps(x1: np.ndarray, x2: np.ndarray):
    import ml_dtypes

    bf16 = ml_dtypes.bfloat16
    x1 = _to_np(x1).reshape(B, C, N)
    x2 = _to_np(x2).reshape(B, C, N)
    # x1 blocked-transposed: x1t[b, p, j, c] = x1[b, c, j*128 + p]
    x1t = np.ascontiguousarray(
        x1.reshape(B, C, NB, C).transpose(0, 3, 2, 1)
    ).astype(bf16)
    x2b = x2.astype(bf16)
    ident, ind8, bd8 = _consts_np()
    return [
        {"x1t": x1t[i], "x2": x2b[i], "ident": ident, "ind8": ind8, "bd8": bd8}
        for i in range(NCORES)
    ]


def kernel(x1: np.ndarray, x2: np.ndarray) -> np.ndarray:
    from concourse.bass_utils import run_bass_kernel_spmd

    nc = _get_nc()
    in_maps = make_in_maps(x1, x2)
    res = run_bass_kernel_spmd(nc, in_maps, core_ids=list(range(NCORES)))
    outs = []
    for i in range(NCORES):
        o = np.asarray(res.results[i]["out"], dtype=np.float32)  # [128, NB, C]
        outs.append(o.transpose(2, 1, 0).reshape(C, N))          # [C, N]
    return np.stack(outs, axis=0).reshape(B, C, H, W)


# revision 7
# speedup vs baseline: 1.6562x; 1.0820x over previous
"""Trainium2 Bass kernel for cross "efficient attention".

Reference computation (per batch b, head h, with C=128, HEADS=8, hc=16, n=16384):
    k = x2[b].reshape(HEADS, hc, n); v = x1[b].reshape(HEADS, hc, n)
    key_sm   = softmax(k, axis=-1)          # over n
    query_sm = softmax(k, axis=1)           # over hc (head channels)
    context  = key_sm @ v^T                 # (hc, hc)
    out[b,h] = context^T @ query_sm         # (hc, n)

Sharding: data-parallel over batch B=8 across the 8 NeuronCores (no
collectives).  Inputs are cast to bf16 on the host (tolerance is 2e-2;
bf16 end-to-end measures ~4e-3), halving HBM traffic, and x1 is laid
out host-side as [128, N/128, C] blocks so every DMA descriptor is a
contiguous run >= 4 KiB.

Per-core pipeline (N = 16384 in 4 slabs of 4096 = 32 chunks of 128):
  pass 1 (per chunk j):
    MM_t : transpose-mode matmul -> te PSUM bf16 (grouped 8 chunks/tile)
    MM_cs: matmul(lhsT=exp_chunk, rhs=ind8) -> per-slab PSUM f32
           accumulator [128, 32*8] (per-head colsums, transposed layout)
    one wide vector copy te -> eT slab buffer per group (bf16->bf16 2x)
    MM_ctx (one slab lag, interleaved per group): ctx += eT^T @ vT
  per slab: one reciprocal_approx_fast [128,256] PSUM->SBUF (rcp_all)
  bd = (ctx / rowsum) * blockdiag_mask  (bf16 [C,C])
  pass 2 (per 2048 block, chunk j): MM_att: matmul(lhsT=exp_chunk,
    rhs=bd) -> attT PSUM f32 [128, 2048]; one vector tensor_mul per
    block with the per-head reciprocals broadcast via a stride-0 AP ->
    bf16 out tile -> DMA out (ACT HWDGE ring).
Output leaves the device transposed ([128, N/128, C] blocks); the host
reassembles [C, H, W].
"""

import numpy as np
from contextlib import ExitStack

B, C, H, W = 8, 128, 128, 128
N = H * W                 # 16384
HEADS, HC = 8, 16
NCORES = 8
SLAB = 4096               # pass-1 slab width (32 chunks of 128)
NCH = SLAB // C           # chunks per slab = 32
NSLAB = N // SLAB         # 4
NB = N // C               # 128 chunk-blocks total
GRP = 8                   # transpose chunks batched per PSUM group tile
NGRP = NCH // GRP         # copy groups per slab = 4
OB = 2048                 # pass-2 output block width
NOB = N // OB             # 8
OCH = OB // C             # chunks per output block = 16

_cache: dict = {}


def _build():
    import concourse.bass as bass
    import concourse.tile as tile
    from concourse import bacc, mybir

    FP32 = mybir.dt.float32
    BF16 = mybir.dt.bfloat16
    AF = mybir.ActivationFunctionType

    nc = bacc.Bacc("TRN2", target_bir_lowering=False, debug=False)

    x1t_d = nc.dram_tensor("x1t", [C, NB, C], BF16, kind="ExternalInput")
    x2_d = nc.dram_tensor("x2", [C, N], BF16, kind="ExternalInput")
    id_d = nc.dram_tensor("ident", [C, C], BF16, kind="ExternalInput")
    ind8_d = nc.dram_tensor("ind8", [C, HEADS], BF16, kind="ExternalInput")
    bd8_d = nc.dram_tensor("bd8", [C, C], BF16, kind="ExternalInput")
    out_d = nc.dram_tensor("out", [C, NB, C], BF16, kind="ExternalOutput")

    with tile.TileContext(nc) as tc:
        with ExitStack() as ctx:
            persist = ctx.enter_context(tc.tile_pool(name="persist", bufs=1))
            x2ld = ctx.enter_context(tc.tile_pool(name="x2ld", bufs=2))
            vTp = ctx.enter_context(tc.tile_pool(name="vTp", bufs=2))
            eTp = ctx.enter_context(tc.tile_pool(name="eTp", bufs=2))
            outp = ctx.enter_context(tc.tile_pool(name="outp", bufs=2))
            smalls = ctx.enter_context(tc.tile_pool(name="smalls", bufs=1))

            exp_nat = persist.tile([C, N], BF16, tag="exp_nat")
            rcp_all = persist.tile([C, NB * HEADS], FP32, tag="rcp_all")
            rs_acc = smalls.tile([C, NSLAB], FP32, tag="rs_acc")
            ident = smalls.tile([C, C], BF16, tag="ident")
            ind8 = smalls.tile([C, HEADS], BF16, tag="ind8")
            bd8 = smalls.tile([C, C], BF16, tag="bd8")
            bd = smalls.tile([C, C], BF16, tag="bd")

            with tc.tile_pool(name="psctx", bufs=1, space="PSUM") as ps_ctx, \
                 tc.tile_pool(name="pstre", bufs=4, space="PSUM") as ps_te, \
                 tc.tile_pool(name="pscs", bufs=2, space="PSUM") as ps_cs:
                ctx_ps = ps_ctx.tile([C, C], FP32, tag="ctx")

                n_mm = NB
                mm_idx = 0

                def emit_ctx(eT, vT, g):
                    # ctx matmuls for chunk group g of the previous slab
                    nonlocal mm_idx
                    eTv = eT[:].rearrange("p (j c) -> p j c", c=C)
                    vTv = vT[:].rearrange("p (j c) -> p j c", c=C)
                    for j in range(g * GRP, (g + 1) * GRP):
                        nc.tensor.matmul(
                            ctx_ps[:],
                            eTv[:, j, :],       # lhsT: (n0=128, c_k=128)
                            vTv[:, j, :],       # rhs : (n0=128, c_v=128)
                            start=(mm_idx == 0),
                            stop=(mm_idx == n_mm - 1),
                        )
                        mm_idx += 1

                nc.scalar.dma_start(out=ident[:], in_=id_d[:])
                nc.scalar.dma_start(out=ind8[:], in_=ind8_d[:])
                nc.scalar.dma_start(out=bd8[:], in_=bd8_d[:])
                pending = None   # (eT, vT) of the previous slab
                for i in range(NSLAB):
                    sl = bass.ds(i * SLAB, SLAB)
                    x2t = x2ld.tile([C, SLAB], BF16, tag="x2t")
                    nc.sync.dma_start(out=x2t[:], in_=x2_d[:, sl])
                    vT = vTp.tile([C, SLAB], BF16, tag="vT")
                    nc.sync.dma_start(
                        out=vT[:].rearrange("p (j c) -> p j c", c=C),
                        in_=x1t_d[:, bass.ds(i * NCH, NCH), :],
                    )

                    nc.scalar.activation(
                        exp_nat[:, sl], x2t[:], AF.Exp,
                        accum_out=rs_acc[:, i:i + 1],
                    )

                    # per-chunk transpose (bf16, grouped PSUM tiles) +
                    # colsum (f32 accum tile); one wide copy per group;
                    # prev-slab ctx matmuls interleaved per group
                    eT = eTp.tile([C, SLAB], BF16, tag="eT")
                    cs_ps = ps_cs.tile([C, NCH * HEADS], FP32, tag="cs")
                    for g in range(NGRP):
                        te = ps_te.tile([C, GRP * C], BF16, tag="te")
                        for jj in range(GRP):
                            j = g * GRP + jj
                            e_chunk = exp_nat[:, bass.ds(i * SLAB + j * C, C)]
                            nc.tensor.transpose(
                                te[:, bass.ds(jj * C, C)], e_chunk, ident[:]
                            )
                            nc.tensor.matmul(
                                cs_ps[:, bass.ds(j * HEADS, HEADS)],
                                e_chunk, ind8[:],
                            )
                        nc.vector.tensor_copy(
                            eT[:, bass.ds(g * GRP * C, GRP * C)], te[:]
                        )
                        if pending is not None:
                            emit_ctx(*pending, g)

                    # per-slab: one reciprocal over the colsum accumulator
                    nc.vector.reciprocal_approx_fast(
                        out=rcp_all[:, bass.ds(i * NCH * HEADS, NCH * HEADS)],
                        in_=cs_ps[:],
                    )
                    pending = (eT, vT)
                for g in range(NGRP):
                    emit_ctx(*pending, g)

                # ---- block-diagonal context weights ----
                rowsum = smalls.tile([C, 1], FP32, tag="rowsum")
                nc.vector.tensor_reduce(
                    rowsum[:], rs_acc[:], mybir.AxisListType.X, mybir.AluOpType.add
                )
                rs_rcp = smalls.tile([C, 1], FP32, tag="rs_rcp")
                nc.vector.reciprocal(rs_rcp[:], rowsum[:])
                scaled = smalls.tile([C, C], BF16, tag="scaled")
                nc.vector.tensor_scalar(
                    scaled[:], ctx_ps[:], rs_rcp[:, 0:1], None, mybir.AluOpType.mult
                )
                nc.vector.tensor_mul(bd[:], scaled[:], bd8[:])

            # ---- pass 2: attended (transposed), normalize, store ----
            with tc.tile_pool(name="psatt", bufs=2, space="PSUM") as ps_att:
                for b in range(NOB):
                    att = ps_att.tile([C, OB], FP32, tag="att")
                    for j in range(OCH):
                        nc.tensor.matmul(
                            att[:, bass.ds(j * C, C)],
                            exp_nat[:, bass.ds(b * OB + j * C, C)],
                            bd[:],
                        )
                    ot = outp.tile([C, OB], BF16, tag="ot")
                    nc.vector.tensor_mul(
                        ot[:].rearrange("p (j h c) -> p j h c", h=HEADS, c=HC),
                        att[:].rearrange("p (j h c) -> p j h c", h=HEADS, c=HC),
                        rcp_all[:, bass.ds(b * OCH * HEADS, OCH * HEADS)]
                        .rearrange("p (j h) -> p j h", h=HEADS)
                        .broadcast_to([C, OCH, HEADS, HC]),
                    )
                    nc.scalar.dma_start(
                        out=out_d[:, bass.ds(b * OCH, OCH), :],
                        in_=ot[:].rearrange("p (j c) -> p j c", c=C),
                    )

    nc.compile()
    return nc


def _get_nc():
    if "nc" not in _cache:
        _cache["nc"] = _build()
    return _cache["nc"]


def _consts_np():
    import ml_dtypes

    bf16 = ml_dtypes.bfloat16
    ident = np.eye(C, dtype=np.float32).astype(bf16)
    ind8 = np.zeros((C, HEADS), dtype=np.float32)
    for h in range(HEADS):
        ind8[h * HC:(h + 1) * HC, h] = 1.0
    bd8 = np.zeros((C, C), dtype=np.float32)
    for h in range(HEADS):
        bd8[h * HC:(h + 1) * HC, h * HC:(h + 1) * HC] = 1.0
    return ident, ind8.astype(bf16), bd8.astype(bf16)


def _to_np(a) -> np.ndarray:
    """Materialize to float32 numpy; retry once on a transient bad fetch
    (device-backed arrays have been observed to materialize NaNs once)."""
    out = np.asarray(a, dtype=np.float32)
    if np.isnan(out).any():
        out = np.asarray(a, dtype=np.float32)
    return out


def make_in_maps(x1: np.ndarray, x2: np.ndarray):
    import ml_dtypes

    bf16 = ml_dtypes.bfloat16
    x1 = _to_np(x1).reshape(B, C, N)
    x2 = _to_np(x2).reshape(B, C, N)
    # x1 blocked-transposed: x1t[b, p, j, c] = x1[b, c, j*128 + p]
    x1t = np.ascontiguousarray(
        x1.reshape(B, C, NB, C).transpose(0, 3, 2, 1)
    ).astype(bf16)
    x2b = x2.astype(bf16)
    ident, ind8, bd8 = _consts_np()
    return [
        {"x1t": x1t[i], "x2": x2b[i], "ident": ident, "ind8": ind8, "bd8": bd8}
        for i in range(NCORES)
    ]


def kernel(x1: np.ndarray, x2: np.ndarray) -> np.ndarray:
    from concourse.bass_utils import run_bass_kernel_spmd

    nc = _get_nc()
    in_maps = make_in_maps(x1, x2)
    res = run_bass_kernel_spmd(nc, in_maps, core_ids=list(range(NCORES)))
    outs = []
    for i in range(NCORES):
        o = np.asarray(res.results[i]["out"], dtype=np.float32)  # [128, NB, C]
        outs.append(o.transpose(2, 1, 0).reshape(C, N))          # [C, N]
    return np.stack(outs, axis=0).reshape(B, C, H, W)
